# revision 53
# baseline (speedup 1.0000x reference)
"""MatchLSTM attention kernel for 8 Trainium2 NeuronCores.

Reference computation (B=64, T=2048, D=512):
    G   = tanh(input_p@Wp.T + bp + input_q@Wq.T + bq + h_tm1@Wr.T + br)
    a   = softmax(G@w + match_b)            over T
    z   = sum_t a[:,t] * input_q[:,t,:]
    out = concat([input_p, z], -1)

Sharding: data-parallel over batch, 8 batches per core, weights replicated.

Per-core device pipeline:
  - c^T[o,b] = (Wp.T;Wr.T;bias) matmuls against (ip^T;h^T;ones)  [once]
  - G^T[o,tok] tiles via fp8e4 DoubleRow matmuls (K=256 per instr, 0.5
    cyc/row): Wq^T-chunks (stationary) x X^T-chunks (moving), fp32 PSUM
  - tanh on ScalarE with per-partition bias c^T -> bf16 SBUF  [ACT is the
    bottleneck engine: 8192 free-elems/batch at 0.833 ns/elem]
  - scores come out TRANSPOSED for free: lhsT = tanh-tile [o,128 toks],
    rhs = w-chunk [o,1] -> sT[128,1] per token chunk (1-row matmuls,
    PSUM-accumulated over the 4 o-chunks; each column's accumulation
    group runs start->stop without interleaving other groups in its
    PSUM bank -- interleaved groups corrupt accumulation on HW)
  - exp on sT -> bf16 esc, one instruction per PAIR of batches (halves
    the ACT per-instruction init overhead; the last two batches retire
    singly so the final z stays off the drain tail); sumexp via an
    all-ones matmul (per-chunk partition sums broadcast to every
    partition) + a free-dim reduce on VectorE
  - z^T[q,1] per q-chunk: lhsT = X-natural chunk [tok,128 q] (fp8),
    rhs = esc[:,j] (1-row matmuls accumulated over the 16 token chunks)
  - reciprocal + scale on VectorE, DMA out.
    Softmax max-subtraction skipped: |s| <= sum|w|+1 < 25, exp safe.
  - X is staged twice from host (fp8 transposed for G, fp8 natural for z)
    so no DMA-transpose is needed; scores/exp/z of batch b-1 are emitted
    inside batch b's G phase so ACT never idles.
"""

import sys

if "/opt/trn_rl_repo" not in sys.path:
    sys.path.insert(0, "/opt/trn_rl_repo")

import numpy as np
import ml_dtypes

N_CORES = 8
B, T, D = 64, 2048, 512
PB = B // N_CORES          # batches per core
KC = D // 128              # 4 contraction / o / q chunks of 128
NJ = T // 128              # 16 token chunks of 128
CROWS = 2 * D              # cw/cx rows: Wp.T, Wr.T (bias folded in at the
                           # cT copy, keeping the startup cw DMA minimal)
NKC = CROWS // 128         # 8 contraction chunks for the c matmuls

BF16 = ml_dtypes.bfloat16
FP8 = ml_dtypes.float8_e4m3

CW_FP8 = True  # c-projection weights in fp8 (faster startup DMA)

_CACHE: dict = {}


def _build_program():
    import concourse.bacc as bacc
    import concourse.tile as tile
    import concourse.mybir as mybir
    from concourse.bass import MemorySpace

    dt = mybir.dt
    F32 = dt.float32
    BF = dt.bfloat16
    F8 = dt.float8e4
    AF = mybir.ActivationFunctionType
    DR = mybir.MatmulPerfMode.DoubleRow

    nc = bacc.Bacc(
        "TRN2", target_bir_lowering=False, debug=False, num_devices=N_CORES
    )

    xt_d = nc.dram_tensor("xt8", [PB, D, T], F8, kind="ExternalInput")     # X^T
    xn_d = nc.dram_tensor("xn8", [PB, T, D], F8, kind="ExternalInput")     # X
    wq_d = nc.dram_tensor("wq8", [D, D], F8, kind="ExternalInput")         # Wq.T [q,o]
    CWD = F8 if CW_FP8 else BF
    cw_d = nc.dram_tensor("cw", [CROWS, D], CWD, kind="ExternalInput")     # [Wp.T;Wr.T;bias;0]
    # cx/wcol pre-rearranged on host to 128-partition-major so their DMAs
    # move whole per-partition rows (tiny-row DMAs pay a 7ns/descriptor
    # floor: 504ns for cx in [CROWS, PB] layout vs 56ns this way)
    cx_d = nc.dram_tensor("cx", [128, NKC * PB], CWD, kind="ExternalInput")
    bias_d = nc.dram_tensor("bias", [128, KC], F32, kind="ExternalInput")
    wcol_d = nc.dram_tensor("wcol", [128, KC], BF, kind="ExternalInput")
    z_d = nc.dram_tensor("z", [128, PB * KC], F32, kind="ExternalOutput")  # z^T chunks

    with tile.TileContext(nc) as tc:
        with (
            tc.tile_pool(name="consts", bufs=1) as consts,
            tc.tile_pool(name="xT_p", bufs=3) as xT_pool,
            tc.tile_pool(name="xn_p", bufs=5) as xn_pool,
            tc.tile_pool(name="th_p", bufs=10) as th_pool,
            tc.tile_pool(name="esc_p", bufs=2) as esc_pool,
            tc.tile_pool(name="small_p", bufs=2) as small_pool,
            tc.tile_pool(name="zout_p", bufs=1) as zout_pool,
            tc.tile_pool(name="pG", bufs=3, space=MemorySpace.PSUM) as pG,
            tc.tile_pool(name="pSZ", bufs=2, space=MemorySpace.PSUM) as pSZ,
        ):
            # ---- constants + batch-0 DMAs, interleaved so the critical path
            # to the first tanh (wq8+xT0[t0:512] for G, then cw+cx for the
            # bias overlapping G's compute) clears the serialized DMA device
            # as early as possible ------------------------------------------
            wq_s = consts.tile([128, KC, D], F8, tag="wq", name="wq_s")
            nc.sync.dma_start(out=wq_s, in_=wq_d.rearrange("(c p) o -> p c o", p=128))
            xT0 = xT_pool.tile([128, KC, T], F8, tag="xT", name="xT")
            nc.sync.dma_start(
                out=xT0[:, :, 0:512],
                in_=xt_d[0, :, 0:512].rearrange("(c p) t -> p c t", p=128),
            )
            cw_s = consts.tile([128, NKC, D], F8 if CW_FP8 else BF, tag="cw", name="cw_s")
            nc.sync.dma_start(out=cw_s, in_=cw_d.rearrange("(c p) o -> p c o", p=128))
            cx_s = consts.tile([128, NKC, PB], F8 if CW_FP8 else BF, tag="cx", name="cx_s")
            nc.sync.dma_start(out=cx_s, in_=cx_d[:, :])
            bias_s = consts.tile([128, KC], F32, tag="bias", name="bias_s")
            nc.sync.dma_start(out=bias_s, in_=bias_d[:, :])
            nc.sync.dma_start(
                out=xT0[:, :, 512:1024],
                in_=xt_d[0, :, 512:1024].rearrange("(c p) t -> p c t", p=128),
            )
            wcol_s = consts.tile([128, KC], BF, tag="wcol", name="wcol_s")
            nc.sync.dma_start(out=wcol_s, in_=wcol_d[:, :])
            nc.sync.dma_start(
                out=xT0[:, :, 1024:2048],
                in_=xt_d[0, :, 1024:2048].rearrange("(c p) t -> p c t", p=128),
            )
            ones_bf = consts.tile([128, 128], BF, tag="ones", name="ones_bf")
            nc.vector.memset(ones_bf, 1.0)
            # warm the ACT table set (tanh/exp share one set)
            dummy_s = consts.tile([1, 1], F32, tag="dummy", name="dummy_s")
            nc.scalar.activation(
                out=dummy_s, in_=ones_bf[0:1, 0:1], func=AF.Tanh, bias=0.0, scale=1.0
            )

            # batch-0's first 512-token G block goes ahead of the c-matmuls
            # in the PE queue: its xT data lands before cw does, and PE is
            # in-order (the i=1 half would wait on a later DMA, so it stays
            # in the batch loop behind the c-matmuls)
            g0 = pG.tile([128, 1024], F32, tag="g", name="g_ps")
            th0 = th_pool.tile([128, 1024], BF, tag="th", name="th")
            for kk in range(KC // 2):
                nc.tensor.matmul(
                    g0[:, 0:512],
                    wq_s[:, 2 * kk : 2 * kk + 2, 0:128],
                    xT0[:, 2 * kk : 2 * kk + 2, 0:512],
                    start=(kk == 0),
                    stop=(kk == KC // 2 - 1),
                    perf_mode=DR,
                )

            # ---- c^T[o, b] for all batches (once); per-oc copy so the
            # first tanh's bias only waits on cw chunk 0 --------------------
            c_ps = pSZ.tile([128, KC, PB], F32, tag="sz", name="c_ps")
            cT_s = consts.tile([128, KC, PB], F32, tag="cT", name="cT_s")
            for oc in range(KC):
                for k in range(NKC):
                    nc.tensor.matmul(
                        c_ps[:, oc, :],
                        cw_s[:, k, oc * 128 : (oc + 1) * 128],
                        cx_s[:, k, :],
                        start=(k == 0),
                        stop=(k == NKC - 1),
                    )
                nc.vector.tensor_scalar_add(
                    out=cT_s[:, oc, :], in0=c_ps[:, oc, :],
                    scalar1=bias_s[:, oc : oc + 1],
                )

            zout_s = zout_pool.tile([128, PB, KC], F32, tag="zout", name="zout_s")

            # per-batch state carried across the software pipeline
            xT = [None] * PB      # [128, KC, T] fp8 transposed X
            xn = [None] * PB      # [128, NJ, D] fp8 natural X
            th = [[None] * (2 * KC) for _ in range(PB)]  # tanh tiles (h*KC+oc)
            NP = PB // 2          # exp/sumexp run once per PAIR of batches
            szp = [None] * NP     # [128,2,37] psum: sT 0:16, colsum 16:32, zT 32:36
            esc = [None] * NP     # [128, 2, NJ] exp scores per pair
            rse = [None] * NP     # [128, 2] f32 1/sumexp per pair
            xT[0] = xT0

            def dma_xt(b):
                xT[b] = xT_pool.tile([128, KC, T], F8, tag="xT", name="xT")
                nc.sync.dma_start(
                    out=xT[b], in_=xt_d[b].rearrange("(c p) t -> p c t", p=128)
                )

            def dma_xn(b):
                xn[b] = xn_pool.tile([128, NJ, D], F8, tag="xn", name="xn")
                nc.sync.dma_start(
                    out=xn[b], in_=xn_d[b].rearrange("(j p) q -> p j q", p=128)
                )

            def emit_g_tanh(b, h, oc, split=False):
                if b == 0 and h == 0 and oc == 0:
                    # first half's G ran ahead of the c-matmuls; tanh per 512
                    # tokens so ACT starts as soon as the bias lands
                    nc.scalar.activation(
                        out=th0[:, 0:512], in_=g0[:, 0:512], func=AF.Tanh,
                        bias=cT_s[:, 0, 0:1], scale=1.0,
                    )
                    for kk in range(KC // 2):
                        nc.tensor.matmul(
                            g0[:, 512:1024],
                            wq_s[:, 2 * kk : 2 * kk + 2, 0:128],
                            xT0[:, 2 * kk : 2 * kk + 2, 512:1024],
                            start=(kk == 0),
                            stop=(kk == KC // 2 - 1),
                            perf_mode=DR,
                        )
                    nc.scalar.activation(
                        out=th0[:, 512:1024], in_=g0[:, 512:1024], func=AF.Tanh,
                        bias=cT_s[:, 0, 0:1], scale=1.0,
                    )
                    th[0][0] = th0
                    return
                g = pG.tile([128, 1024], F32, tag="g", name="g_ps")
                t = th_pool.tile([128, 1024], BF, tag="th", name="th")
                # i-outer so each 512-wide PSUM region's accumulation group
                # (kk 0->1) runs start->stop consecutively
                for i in range(2):
                    for kk in range(KC // 2):
                        t0 = h * 1024 + i * 512
                        nc.tensor.matmul(
                            g[:, i * 512 : (i + 1) * 512],
                            wq_s[:, 2 * kk : 2 * kk + 2, oc * 128 : (oc + 1) * 128],
                            xT[b][:, 2 * kk : 2 * kk + 2, t0 : t0 + 512],
                            start=(kk == 0),
                            stop=(kk == KC // 2 - 1),
                            perf_mode=DR,
                        )
                    if split:
                        # batch-0 startup: tanh per 512 tokens right behind
                        # each PSUM region so ACT starts on partial DMA data
                        nc.scalar.activation(
                            out=t[:, i * 512 : (i + 1) * 512],
                            in_=g[:, i * 512 : (i + 1) * 512], func=AF.Tanh,
                            bias=cT_s[:, oc, b : b + 1], scale=1.0,
                        )
                if not split:
                    nc.scalar.activation(
                        out=t, in_=g, func=AF.Tanh,
                        bias=cT_s[:, oc, b : b + 1], scale=1.0,
                    )
                th[b][h * KC + oc] = t

            def emit_scores(b, h):
                p, i = b // 2, b % 2
                if szp[p] is None:
                    szp[p] = pSZ.tile([128, 2, 37], F32, tag="sz", name="sz_ps")
                for jj in range(8):
                    j = h * 8 + jj
                    for oc in range(KC):
                        nc.tensor.matmul(
                            szp[p][:, i, j : j + 1],
                            th[b][h * KC + oc][:, jj * 128 : (jj + 1) * 128],
                            wcol_s[:, oc : oc + 1],
                            start=(oc == 0),
                            stop=(oc == KC - 1),
                            skip_group_check=True,
                        )

            def emit_exp(p, single=None):
                # one exp instruction covers both batches of the pair
                # (or just one for the trailing single batches, keeping the
                # final batch's z off the drain tail)
                w2 = 1 if single is not None else 2
                i0 = 0 if single is None else single % 2
                esc[p] = esc_pool.tile([128, w2, NJ], BF, tag="esc", name="esc")
                nc.scalar.activation(
                    out=esc[p], in_=szp[p][:, i0 : i0 + w2, 0:NJ], func=AF.Exp,
                    bias=0.0, scale=1.0,
                )
                # sumexp: per-chunk partition sums on PE (every out partition
                # gets all 16 chunk sums), then a free-dim reduce on DVE
                for i in range(w2):
                    nc.tensor.matmul(
                        szp[p][:, i0 + i, 16:32], ones_bf, esc[p][:, i, :],
                        start=True, stop=True, skip_group_check=True,
                    )
                sesb = small_pool.tile([128, w2], F32, tag="sesb", name="sesb")
                nc.vector.tensor_reduce(
                    out=sesb, in_=szp[p][:, i0 : i0 + w2, 16:32],
                    axis=mybir.AxisListType.X, op=mybir.AluOpType.add,
                )
                rse[p] = small_pool.tile([128, w2], F32, tag="rse", name="rse")
                nc.vector.reciprocal(out=rse[p], in_=sesb)

            def emit_z_tail(b, single=False):
                p, i = b // 2, b % 2
                ei = 0 if single else i  # esc/rse column for this batch
                # qc outer: each PSUM column's accumulation group must run
                # start->stop without another group opening in the same bank
                for qc in range(KC):
                    for j in range(NJ):
                        nc.tensor.matmul(
                            szp[p][:, i, 32 + qc : 33 + qc],
                            xn[b][:, j, qc * 128 : (qc + 1) * 128],
                            esc[p][:, ei, j : j + 1],
                            start=(j == 0),
                            stop=(j == NJ - 1),
                            skip_group_check=True,
                        )
                nc.vector.tensor_scalar_mul(
                    out=zout_s[:, b, :], in0=szp[p][:, i, 32:36],
                    scalar1=rse[p][:, ei : ei + 1],
                )

            # ---- software-pipelined batch loop ----------------------------
            # xT(b+1) is prefetched ahead of xn(b): the next batch's G
            # matmuls gate the ACT stream, while z consumes xn only at the
            # end of a batch
            dma_xt(1)
            dma_xn(0)
            dma_xn(1)
            for b in range(PB):
                if b + 2 < PB:
                    dma_xt(b + 2)
                    dma_xn(b + 2)
                # half 0: batches 0..5 retire as pairs (one exp each),
                # batches 6,7 retire singly so the last z stays off the tail
                for oc in range(KC):
                    emit_g_tanh(b, 0, oc, split=(b == 0 and oc == 0))
                    if oc == 1 and b > 0:
                        emit_scores(b - 1, 1)
                        if b - 1 in (1, 3, 5):
                            emit_exp((b - 1) // 2)
                        elif b - 1 == 6:
                            emit_exp(3, single=6)
                    if oc == 3 and b > 0:
                        if b - 1 in (1, 3, 5):
                            emit_z_tail(b - 2)
                            emit_z_tail(b - 1)
                        elif b - 1 == 6:
                            emit_z_tail(6, single=True)
                # half 1
                for oc in range(KC):
                    emit_g_tanh(b, 1, oc)
                    if oc == 1:
                        emit_scores(b, 0)
            # drain the pipeline for the last batch
            emit_scores(PB - 1, 1)
            emit_exp(3, single=7)
            emit_z_tail(PB - 1, single=True)

            nc.sync.dma_start(out=z_d[:, :], in_=zout_s)

    nc.compile()
    return nc


def _get_program():
    if "nc" not in _CACHE:
        _CACHE["nc"] = _build_program()
    return _CACHE["nc"]


def kernel(**inputs) -> np.ndarray:
    from concourse import bass_utils

    inp = {k: np.asarray(v) for k, v in inputs.items()}
    input_p = inp["input_p"].astype(np.float32)
    input_q = inp["input_q"].astype(np.float32)
    h_tm1 = inp["h_tm1"].astype(np.float32)
    Wp, Wq, Wr = inp["Wp"], inp["Wq"], inp["Wr"]
    bp, bq, br = inp["bp"], inp["bq"], inp["br"]
    w = inp["w"]

    # shared (weight) tensors
    CWDT = FP8 if CW_FP8 else BF16
    wq8 = np.ascontiguousarray(Wq.T).astype(FP8)
    cw = np.zeros((CROWS, D), dtype=CWDT)
    cw[:D] = Wp.T.astype(CWDT)
    cw[D : 2 * D] = Wr.T.astype(CWDT)
    bsum = (bp.astype(np.float32) + bq.astype(np.float32) + br.astype(np.float32))
    bias_arr = np.ascontiguousarray(bsum.reshape(KC, 128).T).astype(np.float32)
    # wcol partition-major: wcol[p, c] = w[c*128 + p]
    wcol = np.ascontiguousarray(w.reshape(KC, 128).T).astype(BF16)

    nc = _get_program()

    in_maps = []
    for c in range(N_CORES):
        s = slice(c * PB, (c + 1) * PB)
        cx = np.zeros((CROWS, PB), dtype=CWDT)
        cx[:D] = input_p[s].T.astype(CWDT)
        cx[D : 2 * D] = h_tm1[s].T.astype(CWDT)
        # partition-major: cx2[p, c*PB+b] = cx[c*128+p, b]
        cx = np.ascontiguousarray(
            cx.reshape(NKC, 128, PB).transpose(1, 0, 2).reshape(128, NKC * PB)
        )
        xn8 = input_q[s].astype(FP8)
        xt8 = np.ascontiguousarray(xn8.transpose(0, 2, 1))
        in_maps.append(
            {
                "xt8": xt8,
                "xn8": xn8,
                "wq8": wq8,
                "cw": cw,
                "cx": cx,
                "bias": bias_arr,
                "wcol": wcol,
            }
        )

    res = bass_utils.run_bass_kernel_spmd(
        nc, in_maps, core_ids=list(range(N_CORES))
    )
    zs = []
    for c in range(N_CORES):
        zt = np.asarray(res.results[c]["z"], dtype=np.float32).reshape(128, PB, KC)
        zs.append(zt.transpose(1, 2, 0).reshape(PB, D))
    z = np.concatenate(zs, axis=0)
    return np.concatenate([input_p, z], axis=1)


# revision 56
# speedup vs baseline: 1.0023x; 1.0023x over previous
"""MatchLSTM attention kernel for 8 Trainium2 NeuronCores.

Reference computation (B=64, T=2048, D=512):
    G   = tanh(input_p@Wp.T + bp + input_q@Wq.T + bq + h_tm1@Wr.T + br)
    a   = softmax(G@w + match_b)            over T
    z   = sum_t a[:,t] * input_q[:,t,:]
    out = concat([input_p, z], -1)

Sharding: data-parallel over batch, 8 batches per core, weights replicated.

Per-core device pipeline:
  - c^T[o,b] = (Wp.T;Wr.T;bias) matmuls against (ip^T;h^T;ones)  [once]
  - G^T[o,tok] tiles via fp8e4 DoubleRow matmuls (K=256 per instr, 0.5
    cyc/row): Wq^T-chunks (stationary) x X^T-chunks (moving), fp32 PSUM
  - tanh on ScalarE with per-partition bias c^T -> bf16 SBUF  [ACT is the
    bottleneck engine: 8192 free-elems/batch at 0.833 ns/elem]
  - scores come out TRANSPOSED for free: lhsT = tanh-tile [o,128 toks],
    rhs = w-chunk [o,1] -> sT[128,1] per token chunk (1-row matmuls,
    PSUM-accumulated over the 4 o-chunks; each column's accumulation
    group runs start->stop without interleaving other groups in its
    PSUM bank -- interleaved groups corrupt accumulation on HW)
  - exp on sT -> bf16 esc, one instruction per PAIR of batches (halves
    the ACT per-instruction init overhead; the last two batches retire
    singly so the final z stays off the drain tail); sumexp via an
    all-ones matmul (per-chunk partition sums broadcast to every
    partition) + a free-dim reduce on VectorE
  - z^T[q,1] per q-chunk: lhsT = X-natural chunk [tok,128 q] (fp8),
    rhs = esc[:,j] (1-row matmuls accumulated over the 16 token chunks)
  - reciprocal + scale on VectorE, DMA out.
    Softmax max-subtraction skipped: |s| <= sum|w|+1 < 25, exp safe.
  - X is staged twice from host (fp8 transposed for G, fp8 natural for z)
    so no DMA-transpose is needed; scores/exp/z of batch b-1 are emitted
    inside batch b's G phase so ACT never idles.
"""

import sys

if "/opt/trn_rl_repo" not in sys.path:
    sys.path.insert(0, "/opt/trn_rl_repo")

import numpy as np
import ml_dtypes

N_CORES = 8
B, T, D = 64, 2048, 512
PB = B // N_CORES          # batches per core
KC = D // 128              # 4 contraction / o / q chunks of 128
NJ = T // 128              # 16 token chunks of 128
CROWS = 2 * D              # cw/cx rows: Wp.T, Wr.T (bias folded in at the
                           # cT copy, keeping the startup cw DMA minimal)
NKC = CROWS // 128         # 8 contraction chunks for the c matmuls

BF16 = ml_dtypes.bfloat16
FP8 = ml_dtypes.float8_e4m3

CW_FP8 = True  # c-projection weights in fp8 (faster startup DMA)

_CACHE: dict = {}


def _build_program():
    import concourse.bacc as bacc
    import concourse.tile as tile
    import concourse.mybir as mybir
    from concourse.bass import MemorySpace

    dt = mybir.dt
    F32 = dt.float32
    BF = dt.bfloat16
    F8 = dt.float8e4
    AF = mybir.ActivationFunctionType
    DR = mybir.MatmulPerfMode.DoubleRow

    nc = bacc.Bacc(
        "TRN2", target_bir_lowering=False, debug=False, num_devices=N_CORES
    )

    xt_d = nc.dram_tensor("xt8", [PB, D, T], F8, kind="ExternalInput")     # X^T
    xn_d = nc.dram_tensor("xn8", [PB, T, D], F8, kind="ExternalInput")     # X
    wq_d = nc.dram_tensor("wq8", [D, D], F8, kind="ExternalInput")         # Wq.T [q,o]
    CWD = F8 if CW_FP8 else BF
    cw_d = nc.dram_tensor("cw", [CROWS, D], CWD, kind="ExternalInput")     # [Wp.T;Wr.T;bias;0]
    # cx/wcol pre-rearranged on host to 128-partition-major so their DMAs
    # move whole per-partition rows (tiny-row DMAs pay a 7ns/descriptor
    # floor: 504ns for cx in [CROWS, PB] layout vs 56ns this way)
    cx_d = nc.dram_tensor("cx", [128, NKC * PB], CWD, kind="ExternalInput")
    bias_d = nc.dram_tensor("bias", [128, KC], F32, kind="ExternalInput")
    wcol_d = nc.dram_tensor("wcol", [128, KC], BF, kind="ExternalInput")
    z_d = nc.dram_tensor("z", [128, PB * KC], F32, kind="ExternalOutput")  # z^T chunks

    with tile.TileContext(nc) as tc:
        with (
            tc.tile_pool(name="consts", bufs=1) as consts,
            tc.tile_pool(name="xT_p", bufs=3) as xT_pool,
            tc.tile_pool(name="xn_p", bufs=8) as xn_pool,
            tc.tile_pool(name="th_p", bufs=10) as th_pool,
            tc.tile_pool(name="esc_p", bufs=2) as esc_pool,
            tc.tile_pool(name="small_p", bufs=2) as small_pool,
            tc.tile_pool(name="zout_p", bufs=1) as zout_pool,
            tc.tile_pool(name="pG", bufs=3, space=MemorySpace.PSUM) as pG,
            tc.tile_pool(name="pSZ", bufs=2, space=MemorySpace.PSUM) as pSZ,
        ):
            # ---- constants + batch-0 DMAs, interleaved so the critical path
            # to the first tanh (wq8+xT0[t0:512] for G, then cw+cx for the
            # bias overlapping G's compute) clears the serialized DMA device
            # as early as possible ------------------------------------------
            wq_s = consts.tile([128, KC, D], F8, tag="wq", name="wq_s")
            nc.sync.dma_start(out=wq_s, in_=wq_d.rearrange("(c p) o -> p c o", p=128))
            xT0 = xT_pool.tile([128, KC, T], F8, tag="xT", name="xT")
            nc.sync.dma_start(
                out=xT0[:, :, 0:512],
                in_=xt_d[0, :, 0:512].rearrange("(c p) t -> p c t", p=128),
            )
            cw_s = consts.tile([128, NKC, D], F8 if CW_FP8 else BF, tag="cw", name="cw_s")
            nc.sync.dma_start(out=cw_s, in_=cw_d.rearrange("(c p) o -> p c o", p=128))
            cx_s = consts.tile([128, NKC, PB], F8 if CW_FP8 else BF, tag="cx", name="cx_s")
            nc.sync.dma_start(out=cx_s, in_=cx_d[:, :])
            bias_s = consts.tile([128, KC], F32, tag="bias", name="bias_s")
            nc.sync.dma_start(out=bias_s, in_=bias_d[:, :])
            nc.sync.dma_start(
                out=xT0[:, :, 512:1024],
                in_=xt_d[0, :, 512:1024].rearrange("(c p) t -> p c t", p=128),
            )
            wcol_s = consts.tile([128, KC], BF, tag="wcol", name="wcol_s")
            nc.sync.dma_start(out=wcol_s, in_=wcol_d[:, :])
            nc.sync.dma_start(
                out=xT0[:, :, 1024:2048],
                in_=xt_d[0, :, 1024:2048].rearrange("(c p) t -> p c t", p=128),
            )
            ones_bf = consts.tile([128, 128], BF, tag="ones", name="ones_bf")
            nc.vector.memset(ones_bf, 1.0)
            # warm the ACT table set (tanh/exp share one set)
            dummy_s = consts.tile([1, 1], F32, tag="dummy", name="dummy_s")
            nc.scalar.activation(
                out=dummy_s, in_=ones_bf[0:1, 0:1], func=AF.Tanh, bias=0.0, scale=1.0
            )

            # batch-0's first 512-token G block goes ahead of the c-matmuls
            # in the PE queue: its xT data lands before cw does, and PE is
            # in-order (the i=1 half would wait on a later DMA, so it stays
            # in the batch loop behind the c-matmuls)
            g0 = pG.tile([128, 1024], F32, tag="g", name="g_ps")
            th0 = th_pool.tile([128, 1024], BF, tag="th", name="th")
            for kk in range(KC // 2):
                nc.tensor.matmul(
                    g0[:, 0:512],
                    wq_s[:, 2 * kk : 2 * kk + 2, 0:128],
                    xT0[:, 2 * kk : 2 * kk + 2, 0:512],
                    start=(kk == 0),
                    stop=(kk == KC // 2 - 1),
                    perf_mode=DR,
                )

            # ---- c^T[o, b] for all batches (once); per-oc copy so the
            # first tanh's bias only waits on cw chunk 0 --------------------
            c_ps = pSZ.tile([128, KC, PB], F32, tag="sz", name="c_ps")
            cT_s = consts.tile([128, KC, PB], F32, tag="cT", name="cT_s")
            for oc in range(KC):
                for k in range(NKC):
                    nc.tensor.matmul(
                        c_ps[:, oc, :],
                        cw_s[:, k, oc * 128 : (oc + 1) * 128],
                        cx_s[:, k, :],
                        start=(k == 0),
                        stop=(k == NKC - 1),
                    )
                nc.vector.tensor_scalar_add(
                    out=cT_s[:, oc, :], in0=c_ps[:, oc, :],
                    scalar1=bias_s[:, oc : oc + 1],
                )

            zout_s = zout_pool.tile([128, PB, KC], F32, tag="zout", name="zout_s")

            # per-batch state carried across the software pipeline
            xT = [None] * PB      # [128, KC, T] fp8 transposed X
            xn = [None] * PB      # [128, NJ, D] fp8 natural X
            th = [[None] * (2 * KC) for _ in range(PB)]  # tanh tiles (h*KC+oc)
            # exp/sumexp batching: tiles 0,1 group batches 0-2 and 3-5
            # (one exp instruction each); tile 2 holds batches 6,7 which
            # retire singly so the final z stays off the drain tail
            szp = [None] * 3      # [128,w,37] psum: sT 0:16, colsum 16:32, zT 32:36
            esc = [None] * 3      # [128, w, NJ] exp scores
            rse = [None] * 3      # [128, w] f32 1/sumexp
            expbase = [0, 0, 0]   # first column covered by the tile's live exp

            def bcol(b):
                return (b // 3, b % 3) if b < 6 else (2, b - 6)
            xT[0] = xT0

            def dma_xt(b):
                xT[b] = xT_pool.tile([128, KC, T], F8, tag="xT", name="xT")
                nc.sync.dma_start(
                    out=xT[b], in_=xt_d[b].rearrange("(c p) t -> p c t", p=128)
                )

            def dma_xn(b):
                xn[b] = xn_pool.tile([128, NJ, D], F8, tag="xn", name="xn")
                nc.sync.dma_start(
                    out=xn[b], in_=xn_d[b].rearrange("(j p) q -> p j q", p=128)
                )

            def emit_g_tanh(b, h, oc, split=False):
                if b == 0 and h == 0 and oc == 0:
                    # first half's G ran ahead of the c-matmuls; tanh per 512
                    # tokens so ACT starts as soon as the bias lands
                    nc.scalar.activation(
                        out=th0[:, 0:512], in_=g0[:, 0:512], func=AF.Tanh,
                        bias=cT_s[:, 0, 0:1], scale=1.0,
                    )
                    for kk in range(KC // 2):
                        nc.tensor.matmul(
                            g0[:, 512:1024],
                            wq_s[:, 2 * kk : 2 * kk + 2, 0:128],
                            xT0[:, 2 * kk : 2 * kk + 2, 512:1024],
                            start=(kk == 0),
                            stop=(kk == KC // 2 - 1),
                            perf_mode=DR,
                        )
                    nc.scalar.activation(
                        out=th0[:, 512:1024], in_=g0[:, 512:1024], func=AF.Tanh,
                        bias=cT_s[:, 0, 0:1], scale=1.0,
                    )
                    th[0][0] = th0
                    return
                g = pG.tile([128, 1024], F32, tag="g", name="g_ps")
                t = th_pool.tile([128, 1024], BF, tag="th", name="th")
                # i-outer so each 512-wide PSUM region's accumulation group
                # (kk 0->1) runs start->stop consecutively
                for i in range(2):
                    for kk in range(KC // 2):
                        t0 = h * 1024 + i * 512
                        nc.tensor.matmul(
                            g[:, i * 512 : (i + 1) * 512],
                            wq_s[:, 2 * kk : 2 * kk + 2, oc * 128 : (oc + 1) * 128],
                            xT[b][:, 2 * kk : 2 * kk + 2, t0 : t0 + 512],
                            start=(kk == 0),
                            stop=(kk == KC // 2 - 1),
                            perf_mode=DR,
                        )
                    if split:
                        # batch-0 startup: tanh per 512 tokens right behind
                        # each PSUM region so ACT starts on partial DMA data
                        nc.scalar.activation(
                            out=t[:, i * 512 : (i + 1) * 512],
                            in_=g[:, i * 512 : (i + 1) * 512], func=AF.Tanh,
                            bias=cT_s[:, oc, b : b + 1], scale=1.0,
                        )
                if not split:
                    nc.scalar.activation(
                        out=t, in_=g, func=AF.Tanh,
                        bias=cT_s[:, oc, b : b + 1], scale=1.0,
                    )
                th[b][h * KC + oc] = t

            def emit_scores(b, h):
                p, i = bcol(b)
                if szp[p] is None:
                    w = 3 if p < 2 else 2
                    szp[p] = pSZ.tile([128, w, 37], F32, tag="sz", name="sz_ps")
                for jj in range(8):
                    j = h * 8 + jj
                    for oc in range(KC):
                        nc.tensor.matmul(
                            szp[p][:, i, j : j + 1],
                            th[b][h * KC + oc][:, jj * 128 : (jj + 1) * 128],
                            wcol_s[:, oc : oc + 1],
                            start=(oc == 0),
                            stop=(oc == KC - 1),
                            skip_group_check=True,
                        )

            def emit_exp(p, i0, w2):
                # one exp instruction covers columns [i0, i0+w2) of szp[p]
                expbase[p] = i0
                esc[p] = esc_pool.tile([128, w2, NJ], BF, tag="esc", name="esc")
                nc.scalar.activation(
                    out=esc[p], in_=szp[p][:, i0 : i0 + w2, 0:NJ], func=AF.Exp,
                    bias=0.0, scale=1.0,
                )
                # sumexp: per-chunk partition sums on PE (every out partition
                # gets all 16 chunk sums), then a free-dim reduce on DVE
                for i in range(w2):
                    nc.tensor.matmul(
                        szp[p][:, i0 + i, 16:32], ones_bf, esc[p][:, i, :],
                        start=True, stop=True, skip_group_check=True,
                    )
                sesb = small_pool.tile([128, w2], F32, tag="sesb", name="sesb")
                nc.vector.tensor_reduce(
                    out=sesb, in_=szp[p][:, i0 : i0 + w2, 16:32],
                    axis=mybir.AxisListType.X, op=mybir.AluOpType.add,
                )
                rse[p] = small_pool.tile([128, w2], F32, tag="rse", name="rse")
                nc.vector.reciprocal(out=rse[p], in_=sesb)

            def emit_z_tail(b):
                p, i = bcol(b)
                ei = i - expbase[p]  # esc/rse column for this batch
                # qc outer: each PSUM column's accumulation group must run
                # start->stop without another group opening in the same bank
                for qc in range(KC):
                    for j in range(NJ):
                        nc.tensor.matmul(
                            szp[p][:, i, 32 + qc : 33 + qc],
                            xn[b][:, j, qc * 128 : (qc + 1) * 128],
                            esc[p][:, ei, j : j + 1],
                            start=(j == 0),
                            stop=(j == NJ - 1),
                            skip_group_check=True,
                        )
                nc.vector.tensor_scalar_mul(
                    out=zout_s[:, b, :], in0=szp[p][:, i, 32:36],
                    scalar1=rse[p][:, ei : ei + 1],
                )

            # ---- software-pipelined batch loop ----------------------------
            # xT(b+1) is prefetched ahead of xn(b): the next batch's G
            # matmuls gate the ACT stream, while z consumes xn only at the
            # end of a batch
            dma_xt(1)
            dma_xn(0)
            dma_xn(1)
            for b in range(PB):
                if b + 2 < PB:
                    dma_xt(b + 2)
                    dma_xn(b + 2)
                # half 0: batches 0..5 share one exp at batch 6; their z
                # matmuls are split into two bursts (batch 6 and batch 7)
                # to stay inside PE slack; batches 6,7 retire singly so the
                # last z stays off the drain tail
                for oc in range(KC):
                    emit_g_tanh(b, 0, oc, split=(b == 0 and oc == 0))
                    if oc == 1 and b > 0:
                        emit_scores(b - 1, 1)
                        if b - 1 in (2, 5):
                            emit_exp((b - 1) // 3, 0, 3)
                        elif b - 1 == 6:
                            emit_exp(2, 0, 1)
                    if oc == 3 and b in (3, 6, 7):
                        emit_z_tail({3: 0, 6: 3, 7: 6}[b])
                # half 1: one spread-out z burst per trigger so the PE FIFO
                # never stalls ACT behind a long z block
                for oc in range(KC):
                    emit_g_tanh(b, 1, oc)
                    if oc == 1:
                        emit_scores(b, 0)
                    if oc == 2 and b in (3, 6):
                        emit_z_tail(b - 2)
                    if oc == 3 and b in (3, 6):
                        emit_z_tail(b - 1)
            # drain the pipeline for the last batch
            emit_scores(PB - 1, 1)
            emit_exp(2, 1, 1)
            emit_z_tail(PB - 1)

            nc.sync.dma_start(out=z_d[:, :], in_=zout_s)

    nc.compile()
    return nc


def _get_program():
    if "nc" not in _CACHE:
        _CACHE["nc"] = _build_program()
    return _CACHE["nc"]


def kernel(**inputs) -> np.ndarray:
    from concourse import bass_utils

    inp = {k: np.asarray(v) for k, v in inputs.items()}
    input_p = inp["input_p"].astype(np.float32)
    input_q = inp["input_q"].astype(np.float32)
    h_tm1 = inp["h_tm1"].astype(np.float32)
    Wp, Wq, Wr = inp["Wp"], inp["Wq"], inp["Wr"]
    bp, bq, br = inp["bp"], inp["bq"], inp["br"]
    w = inp["w"]

    # shared (weight) tensors
    CWDT = FP8 if CW_FP8 else BF16
    wq8 = np.ascontiguousarray(Wq.T).astype(FP8)
    cw = np.zeros((CROWS, D), dtype=CWDT)
    cw[:D] = Wp.T.astype(CWDT)
    cw[D : 2 * D] = Wr.T.astype(CWDT)
    bsum = (bp.astype(np.float32) + bq.astype(np.float32) + br.astype(np.float32))
    bias_arr = np.ascontiguousarray(bsum.reshape(KC, 128).T).astype(np.float32)
    # wcol partition-major: wcol[p, c] = w[c*128 + p]
    wcol = np.ascontiguousarray(w.reshape(KC, 128).T).astype(BF16)

    nc = _get_program()

    in_maps = []
    for c in range(N_CORES):
        s = slice(c * PB, (c + 1) * PB)
        cx = np.zeros((CROWS, PB), dtype=CWDT)
        cx[:D] = input_p[s].T.astype(CWDT)
        cx[D : 2 * D] = h_tm1[s].T.astype(CWDT)
        # partition-major: cx2[p, c*PB+b] = cx[c*128+p, b]
        cx = np.ascontiguousarray(
            cx.reshape(NKC, 128, PB).transpose(1, 0, 2).reshape(128, NKC * PB)
        )
        xn8 = input_q[s].astype(FP8)
        xt8 = np.ascontiguousarray(xn8.transpose(0, 2, 1))
        in_maps.append(
            {
                "xt8": xt8,
                "xn8": xn8,
                "wq8": wq8,
                "cw": cw,
                "cx": cx,
                "bias": bias_arr,
                "wcol": wcol,
            }
        )

    res = bass_utils.run_bass_kernel_spmd(
        nc, in_maps, core_ids=list(range(N_CORES))
    )
    zs = []
    for c in range(N_CORES):
        zt = np.asarray(res.results[c]["z"], dtype=np.float32).reshape(128, PB, KC)
        zs.append(zt.transpose(1, 2, 0).reshape(PB, D))
    z = np.concatenate(zs, axis=0)
    return np.concatenate([input_p, z], axis=1)
